# revision 28
# baseline (speedup 1.0000x reference)
"""GCN layer (gather + segment-sum + degree norm) on 8 trn2 NeuronCores.

Sharding: destination nodes across cores (12500/core, padded to 12544).

Phase 1 (per core, node-slice sharded): out-degree counts for the core's
node slice via one-hot (quads of four 32-node sub-windows in one DVE op,
k-innermost layout for the DVE 2x mode) + per-column ones-matmuls into a
[128, 8] PSUM tile (sub-windows packed via tile_position partition
offsets, one column per quad); rsqrt(max(od,1)) via Act Rsqrt batched
over 8 quads; h2 = h * od_r in fp16. h and h2 use a transposed
[P, quad, D] layout so each 8-quad block moves with a single DMA of
2KB-per-partition runs. Host assembles the [100000, 256] fp16 gather
table (col 128 = 1.0 so phase 2's matmul accumulates in-degree free).

Phase 2 (per core, dst-window sharded): windows of 128 dst nodes, edges
grouped per (window, chunk-of-32768-src) padded to K*128 slots with
uniform K across cores (single SPMD NEFF). dma_gather calls merged
across blocks of BW windows per chunk (cuts the ~1-2us/call GPSIMD
descriptor-gen overhead). Per (window, chunk): one-hot [P, 128, K]
(k-innermost fp16, DVE 2x) + K matmuls of onehot^T @ msgs[:,k,0:129]
into PSUM [128, 129]; col 128 accumulates in-degree; out = agg *
rsqrt(max(id,1)) collected per block and stored via one transposed DMA.
"""

import numpy as np

import concourse.bass as bass
import concourse.bacc as bacc
import concourse.mybir as mybir
import concourse.tile as tile
from concourse.bass_utils import run_bass_kernel_spmd

N_SRC = 60000
N_DST = 40000
N_NODES = N_SRC + N_DST
D = 128
C = 8
P = 128
NPC = N_NODES // C           # 12500 true rows per core slice
WN = 128                     # phase-2 dst window
NW = (NPC + WN - 1) // WN    # 98
NPC_PAD = NW * WN            # 12544
WN1 = 32                     # phase-1 counting sub-window
NQ = NPC_PAD // P            # 98 quads of 4 sub-windows
QB = 8                       # phase-1 quads per block
ELEM = 256                   # fp16 row: h2 [0:128] | 1.0 | zeros  (512B)
CHUNK = 25000                # equal chunks; int16 gather idx < 32768
NCHUNK = (N_NODES + CHUNK - 1) // CHUNK
BW = 4                       # phase-2 windows per gather block (= parity mod)
MP_BUFS = 3                  # msgs pool buffers; first MP_BUFS blocks pad idx=0
GC = 5                       # gather sub-call size in 128-slot columns

f32 = mybir.dt.float32
fp16 = mybir.dt.float16
i16 = mybir.dt.int16
F16 = np.float16


# ---------------------------------------------------------------- host packing
def _pack_degree(src_idx):
    """Bucket edges by (core-slice of src, 32-node sub-window). Per quad of 4
    sub-windows: uniform K4 columns each (max over sub-windows and cores,
    min 1); slot values are sub-window-local src (0..31), -1 pad."""
    order = np.argsort(src_idx, kind="stable")
    s_src = src_idx[order]
    core_of = s_src // NPC
    loc = s_src - core_of * NPC
    win_of = loc // WN1
    nw1 = NPC_PAD // WN1  # 392

    counts = np.zeros((C, nw1), dtype=np.int64)
    np.add.at(counts, (core_of, win_of), 1)
    K4s = []
    for q in range(NQ):
        m = counts[:, 4 * q : 4 * q + 4].max()
        K4s.append(max(int(-(-m // P)), 1))

    cols = 4 * sum(K4s)
    srcl = np.full((C, P, cols), -1.0, dtype=np.float32)
    col0 = np.concatenate([[0], 4 * np.cumsum(K4s)])
    starts = np.concatenate([[0], np.cumsum(counts.reshape(-1))])
    for c in range(C):
        for q in range(NQ):
            K4 = K4s[q]
            for j4 in range(4):
                w = 4 * q + j4
                gi = c * nw1 + w
                lo, hi = starts[gi], starts[gi + 1]
                if hi == lo:
                    continue
                g = loc[lo:hi] - w * WN1  # sub-window-local 0..31
                n = hi - lo
                j = np.arange(n)
                srcl[c, j % P, col0[q] + j4 * K4 + j // P] = g
    return K4s, col0, srcl.astype(F16)


def _pack_main(src_idx, dst_idx):
    """Slot layout without per-(window, chunk) column alignment: per (w, ch)
    the slot count n_u = max-over-cores edge count (pad slots idx=0/dst=-1);
    per gather call (block of BW windows, chunk) slots are packed back to
    back; only the call tail is padded to a whole 128-column (idx=0 row-0
    gathers so every slot the PE reads is always written). Columns may
    straddle windows; dstl values are window-local dst + 128*(w % BW) so
    each window's one-hot (vs its parity's iota) zeroes foreign slots.

    Returns meta dict consumed by _build_main_nc.
    """
    assert BW <= 8  # parity values 128*BW + 127 must stay fp16-exact (< 2048)
    order = np.argsort(dst_idx, kind="stable")
    s_src = src_idx[order]
    s_dst = dst_idx[order]
    core_of = s_dst // NPC
    loc = s_dst - core_of * NPC
    win_of = loc // WN
    chunk_of = s_src // CHUNK

    counts = np.zeros((C, NW, NCHUNK), dtype=np.int64)
    np.add.at(counts, (core_of, win_of, chunk_of), 1)
    n_u = counts.max(axis=0)  # [NW, NCHUNK] uniform slot counts
    for w in range(NW):
        if n_u[w].sum() == 0:
            n_u[w, 0] = 1  # guarantee >=1 slot per window (PSUM init)

    blocks = [(b, min(b + BW, NW)) for b in range(0, NW, BW)]

    # layout: per call (block, chunk): windows back to back, tail to x128
    call_cols = np.zeros((len(blocks), NCHUNK), dtype=np.int64)
    call_true = np.zeros((len(blocks), NCHUNK), dtype=np.int64)
    call_kcol0 = np.zeros((len(blocks), NCHUNK), dtype=np.int64)
    call_icol0 = np.zeros((len(blocks), NCHUNK), dtype=np.int64)
    slot_a = np.zeros((NW, NCHUNK), dtype=np.int64)  # call-local slot start
    blk_kcol0 = np.zeros(len(blocks), dtype=np.int64)
    kacc = iacc = 0
    for b, (w_lo, w_hi) in enumerate(blocks):
        blk_kcol0[b] = kacc
        for ch in range(NCHUNK):
            a = 0
            for w in range(w_lo, w_hi):
                slot_a[w, ch] = a
                a += n_u[w, ch]
            cols = -(-a // P)
            call_cols[b, ch] = cols
            call_true[b, ch] = cols * P
            call_kcol0[b, ch] = kacc
            call_icol0[b, ch] = iacc
            kacc += cols
            iacc += cols * 8

    # per (w, ch): global col range [glo, ghi)
    glo = np.zeros((NW, NCHUNK), dtype=np.int64)
    ghi = np.zeros((NW, NCHUNK), dtype=np.int64)
    for b, (w_lo, w_hi) in enumerate(blocks):
        for ch in range(NCHUNK):
            for w in range(w_lo, w_hi):
                if n_u[w, ch] == 0:
                    continue
                a = slot_a[w, ch]
                glo[w, ch] = call_kcol0[b, ch] + a // P
                ghi[w, ch] = call_kcol0[b, ch] + -(-(a + n_u[w, ch]) // P)

    idxs = np.zeros((C, P, iacc), dtype=np.int16)
    dstl = np.full((C, P, kacc), -1.0, dtype=np.float32)

    starts2 = np.concatenate([[0], np.cumsum(counts.sum(axis=2).reshape(-1))])
    # per-core per-(w,ch) edge lists, sorted by src
    for c in range(C):
        per_w = {}
        for w in range(NW):
            gi = c * NW + w
            lo, hi = starts2[gi], starts2[gi + 1]
            g_src = s_src[lo:hi]
            g_dst = loc[lo:hi] - w * WN
            so = np.argsort(g_src, kind="stable")
            per_w[w] = (g_src[so], g_dst[so], g_src[so] // CHUNK)
        for b, (w_lo, w_hi) in enumerate(blocks):
            for ch in range(NCHUNK):
                cols = int(call_cols[b, ch])
                if cols == 0:
                    continue
                nslots = cols * P
                lidx = np.zeros(nslots, dtype=np.int64)
                dl = np.full(nslots, -1.0, dtype=np.float32)
                used = 0
                for w in range(w_lo, w_hi):
                    g_src, g_dst, g_ch = per_w[w]
                    sel = g_ch == ch
                    n = int(sel.sum())
                    a = int(slot_a[w, ch])
                    lidx[a : a + n] = g_src[sel] - ch * CHUNK
                    dl[a : a + n] = g_dst[sel] + WN * (w % BW)
                    used = a + int(n_u[w, ch])
                wr = lidx.astype(np.int16).reshape(cols * 8, 16).T
                ic0 = int(call_icol0[b, ch])
                idxs[c, :, ic0 : ic0 + cols * 8] = np.tile(wr, (8, 1))
                kc0 = int(call_kcol0[b, ch])
                dstl[c, np.arange(nslots) % P,
                     kc0 + np.arange(nslots) // P] = dl
    return {
        "blocks": blocks,
        "n_u": n_u,
        "call_true": call_true,
        "call_cols": call_cols,
        "call_kcol0": call_kcol0,
        "call_icol0": call_icol0,
        "blk_kcol0": blk_kcol0,
        "glo": glo,
        "ghi": ghi,
        "kcols": int(kacc),
        "icols": int(iacc),
        "idxs": idxs,
        "dstl": dstl.astype(F16),
    }


# ---------------------------------------------------------------- bass builders
def _build_degree_nc(K4s, col0, repeat=1):
    nc = bacc.Bacc("TRN2", target_bir_lowering=False)
    cols = int(col0[-1])
    K1max = max(K4s)
    srcl_d = nc.dram_tensor("srcl", [P, cols], fp16, kind="ExternalInput")
    iota_d = nc.dram_tensor("iota1", [P, WN1, K1max], fp16, kind="ExternalInput")
    h_d = nc.dram_tensor("h_slice", [P, NQ, D], fp16, kind="ExternalInput")
    h2_d = nc.dram_tensor("h2s", [P, NQ, D], fp16, kind="ExternalOutput")

    qblocks = [(b, min(b + QB, NQ)) for b in range(0, NQ, QB)]

    with tile.TileContext(nc) as tc:
        with (
            tc.tile_pool(name="cst", bufs=1) as cst,
            tc.tile_pool(name="work", bufs=3) as wk,
            tc.tile_pool(name="hbuf", bufs=2) as hb,
            tc.tile_pool(name="psum", bufs=2, space="PSUM") as ps,
        ):
            srcl = cst.tile([P, cols], fp16)
            nc.sync.dma_start(srcl[:], srcl_d[:])
            iota1 = cst.tile([P, WN1, K1max], fp16)
            nc.sync.dma_start(iota1[:], iota_d[:])
            ones = cst.tile([P, 1], fp16)
            nc.vector.memset(ones[:], 1.0)

            def body(_=None):
                for (q_lo, q_hi) in qblocks:
                    nq = q_hi - q_lo
                    odq = ps.tile([P, QB], f32, space="PSUM", tag="odq")
                    for j in range(nq):
                        q = q_lo + j
                        K4 = K4s[q]
                        c0 = int(col0[q])
                        oh = wk.tile([P, 4, WN1, K4], fp16, tag="oh")
                        nc.vector.tensor_tensor(
                            out=oh[:],
                            in0=srcl[:, c0 : c0 + 4 * K4]
                            .rearrange("p (j k) -> p j k", k=K4)[:, :, None, :]
                            .to_broadcast([P, 4, WN1, K4]),
                            in1=iota1[:, None, :, 0:K4].to_broadcast(
                                [P, 4, WN1, K4]
                            ),
                            op=mybir.AluOpType.is_equal,
                        )
                        for j4 in range(4):
                            for k in range(K4):
                                nc.tensor.matmul(
                                    odq[j4 * WN1 : (j4 + 1) * WN1, j : j + 1],
                                    lhsT=oh[:, j4, :, k],
                                    rhs=ones[:],
                                    start=(k == 0),
                                    stop=(k == K4 - 1),
                                    tile_position=(0, j4 * WN1),
                                )
                    clamped = wk.tile([P, QB], f32, tag="cl")
                    nc.vector.tensor_scalar_max(
                        clamped[:, 0:nq], odq[:, 0:nq], 1.0
                    )
                    sq = wk.tile([P, QB], f32, tag="sq")
                    nc.scalar.activation(
                        sq[:, 0:nq], clamped[:, 0:nq],
                        mybir.ActivationFunctionType.Sqrt,
                    )
                    odr = wk.tile([P, QB], f32, tag="odr")
                    nc.vector.reciprocal(odr[:, 0:nq], sq[:, 0:nq])
                    h_blk = hb.tile([P, QB, D], fp16, tag="hblk")
                    nc.sync.dma_start(
                        h_blk[:, 0:nq, :], h_d[:, q_lo:q_hi, :]
                    )
                    h2_blk = hb.tile([P, QB, D], fp16, tag="h2blk")
                    for j in range(nq):
                        nc.vector.tensor_scalar_mul(
                            h2_blk[:, j, :], h_blk[:, j, :], odr[:, j : j + 1]
                        )
                    nc.sync.dma_start(
                        h2_d[:, q_lo:q_hi, :], h2_blk[:, 0:nq, :]
                    )

            if repeat > 1:
                with tc.For_i(0, repeat, 1):
                    body()
            else:
                body()
    nc.compile()
    return nc


def _build_main_nc(meta, repeat=1, parts="full"):
    nc = bacc.Bacc("TRN2", target_bir_lowering=False)
    blocks = meta["blocks"]
    n_u = meta["n_u"]
    call_cols = meta["call_cols"]
    call_kcol0 = meta["call_kcol0"]
    call_icol0 = meta["call_icol0"]
    blk_kcol0 = meta["blk_kcol0"]
    glo, ghi = meta["glo"], meta["ghi"]
    kcols, icols = meta["kcols"], meta["icols"]
    K2max = int((ghi - glo).max())
    h2_d = nc.dram_tensor("h2", [N_NODES, ELEM], fp16, kind="ExternalInput")
    idx_d = nc.dram_tensor("idxs", [P, icols], i16, kind="ExternalInput")
    dstl_d = nc.dram_tensor("dstl", [P, kcols], fp16, kind="ExternalInput")
    iota_d = nc.dram_tensor(
        "iota2", [P, BW, WN, K2max], fp16, kind="ExternalInput"
    )
    out_d = nc.dram_tensor("out_slice", [P, NW, D], f32, kind="ExternalOutput")

    chunk_rows = [min(CHUNK, N_NODES - ch * CHUNK) for ch in range(NCHUNK)]

    with tile.TileContext(nc) as tc:
        with (
            tc.tile_pool(name="cst", bufs=1) as cst,
            tc.tile_pool(name="msgs", bufs=MP_BUFS) as mp,
            tc.tile_pool(name="work", bufs=4) as wk,
            tc.tile_pool(name="fin", bufs=2) as fb,
            tc.tile_pool(name="psum", bufs=4, space="PSUM") as ps,
        ):
            idxs = cst.tile([P, icols], i16)
            nc.sync.dma_start(idxs[:], idx_d[:])
            dstl = cst.tile([P, kcols], fp16)
            nc.sync.dma_start(dstl[:], dstl_d[:])
            iota2 = cst.tile([P, BW, WN, K2max], fp16)
            nc.sync.dma_start(iota2[:], iota_d[:])

            def body(_=None):
                for b, (w_lo, w_hi) in enumerate(blocks):
                    nwb = w_hi - w_lo
                    blk_k0 = int(blk_kcol0[b])
                    Kblk = int(call_cols[b].sum())
                    msgs = mp.tile([P, Kblk, ELEM], fp16, tag="msgs")
                    for ch in range(NCHUNK):
                        Kcall = int(call_cols[b, ch])
                        if Kcall == 0:
                            continue
                        mk0 = int(call_kcol0[b, ch]) - blk_k0
                        ic0 = int(call_icol0[b, ch])
                        # the gather ucode crashes above ~1.5k idxs/call and
                        # slows past ~640 (Q7 scratch + desc ring); sub-call
                        for j in range(0, Kcall, GC):
                            kc = min(GC, Kcall - j)
                            nc.gpsimd.dma_gather(
                                out_ap=msgs[:, mk0 + j : mk0 + j + kc, :],
                                in_ap=h2_d[
                                    ch * CHUNK : ch * CHUNK + chunk_rows[ch], :
                                ],
                                idxs_ap=idxs[
                                    :, ic0 + j * 8 : ic0 + (j + kc) * 8
                                ],
                                num_idxs=kc * P,
                                num_idxs_reg=kc * P,
                                elem_size=ELEM,
                            )
                    if parts == "gather":
                        continue
                    fin = fb.tile([P, BW, D], f32, tag="fin")
                    for w in range(w_lo, w_hi):
                        Kw = int(
                            sum(
                                int(ghi[w, ch] - glo[w, ch])
                                for ch in range(NCHUNK)
                                if n_u[w, ch] > 0
                            )
                        )
                        acc = ps.tile([WN, D + 1], f32, space="PSUM", tag="acc")
                        kk = 0
                        for ch in range(NCHUNK):
                            if n_u[w, ch] == 0:
                                continue
                            g0 = int(glo[w, ch])
                            K = int(ghi[w, ch] - g0)
                            oh = wk.tile([P, WN, K], fp16, tag="oh")
                            nc.vector.tensor_tensor(
                                out=oh[:],
                                in0=dstl[:, None, g0 : g0 + K].to_broadcast(
                                    [P, WN, K]
                                ),
                                in1=iota2[:, w % BW, :, 0:K],
                                op=mybir.AluOpType.is_equal,
                            )
                            if parts == "onehot":
                                continue
                            for k in range(K):
                                nc.tensor.matmul(
                                    acc[:],
                                    lhsT=oh[:, :, k],
                                    rhs=msgs[:, g0 - blk_k0 + k, 0 : D + 1],
                                    start=(kk == 0),
                                    stop=(kk == Kw - 1),
                                )
                                kk += 1
                        clamped = wk.tile([WN, 1], f32, tag="cl")
                        nc.vector.tensor_scalar_max(
                            clamped[:], acc[:, D : D + 1], 1.0
                        )
                        sq = wk.tile([WN, 1], f32, tag="sq")
                        nc.scalar.activation(
                            sq[:], clamped[:],
                            mybir.ActivationFunctionType.Sqrt,
                        )
                        rsq = wk.tile([WN, 1], f32, tag="rsq")
                        nc.vector.reciprocal(rsq[:], sq[:])
                        nc.vector.tensor_scalar_mul(
                            fin[:, w - w_lo, :], acc[:, 0:D], rsq[:, 0:1]
                        )
                    nc.sync.dma_start(
                        out_d[:, w_lo:w_hi, :], fin[:, 0:nwb, :]
                    )

            if repeat > 1:
                with tc.For_i(0, repeat, 1):
                    body()
            else:
                body()
    nc.compile()
    return nc


# ---------------------------------------------------------------- iota tables
def _iota_table(wn, kmax):
    t = np.broadcast_to(
        np.arange(wn, dtype=np.float32)[:, None], (wn, kmax)
    )
    return np.ascontiguousarray(
        np.broadcast_to(t[None], (P, wn, kmax))
    ).astype(F16)


def _iota_parity_table(kmax):
    """iota2[p, par, w, k] = 128*par + w (window parity encoding)."""
    vals = (
        np.arange(BW, dtype=np.float32)[:, None] * WN
        + np.arange(WN, dtype=np.float32)[None, :]
    )
    t = np.broadcast_to(vals[:, :, None], (BW, WN, kmax))
    return np.ascontiguousarray(
        np.broadcast_to(t[None], (P, BW, WN, kmax))
    ).astype(F16)


# ---------------------------------------------------------------- entry point
def kernel(src_embedding, dst_embedding, src_idx, dst_idx, repeat=1):
    src_embedding = np.asarray(src_embedding, dtype=np.float32)
    dst_embedding = np.asarray(dst_embedding, dtype=np.float32)
    src_idx = np.asarray(src_idx).astype(np.int64)
    dst_idx = np.asarray(dst_idx).astype(np.int64)

    h_full = np.concatenate([src_embedding, dst_embedding], axis=0).astype(F16)

    # ---- launch 1: out-degree rsqrt + table scale on device
    K4s, dcol0, srcl = _pack_degree(src_idx)
    nc1 = _build_degree_nc(K4s, dcol0, repeat=repeat)
    iota1_np = _iota_table(WN1, max(K4s))
    in_maps1 = []
    for c in range(C):
        hs = np.zeros((NPC_PAD, D), dtype=F16)
        hs[:NPC] = h_full[c * NPC : (c + 1) * NPC]
        hst = np.ascontiguousarray(
            hs.reshape(NQ, P, D).transpose(1, 0, 2)
        )
        in_maps1.append(
            {"srcl": np.ascontiguousarray(srcl[c]), "iota1": iota1_np,
             "h_slice": hst}
        )
    res1 = run_bass_kernel_spmd(nc1, in_maps1, core_ids=list(range(C)))
    kernel.last_res1 = res1

    # ---- host glue: assemble fp16 gather table (layout only)
    h2 = np.zeros((N_NODES, ELEM), dtype=F16)
    for c in range(C):
        h2s = res1.results[c]["h2s"].transpose(1, 0, 2).reshape(NPC_PAD, D)
        h2[c * NPC : (c + 1) * NPC, :D] = h2s[:NPC]
    h2[:, D] = np.float16(1.0)

    # ---- launch 2: gather + aggregate + normalize
    meta = _pack_main(src_idx, dst_idx)
    nc2 = _build_main_nc(meta, repeat=repeat)
    iota2_np = _iota_parity_table(int((meta["ghi"] - meta["glo"]).max()))
    in_maps2 = [
        {
            "h2": h2,
            "idxs": np.ascontiguousarray(meta["idxs"][c]),
            "dstl": np.ascontiguousarray(meta["dstl"][c]),
            "iota2": iota2_np,
        }
        for c in range(C)
    ]
    res2 = run_bass_kernel_spmd(nc2, in_maps2, core_ids=list(range(C)))
    kernel.last_res2 = res2
    out = np.concatenate(
        [
            res2.results[c]["out_slice"].transpose(1, 0, 2).reshape(
                NPC_PAD, D
            )[:NPC]
            for c in range(C)
        ],
        axis=0,
    )
    return out.astype(np.float32)
